# revision 15
# baseline (speedup 1.0000x reference)
"""Trainium2 Bass kernel for nn_MultiHeadAttention_88330297410289.

Full-input contract: kernel(**inputs) takes the complete tensors
(hidden_states [32,256,2048], Wq/Wk/Wv/Wo [2048,2048], all fp32) and
returns the full output [32,256,2048] fp32.

Strategy: data-parallel over batch across 8 NeuronCores (4 batches =
1024 tokens per core, no collectives). All matmuls run in fp16 (fp32
PSUM). The four projection GEMMs use one level of Winograd-Strassen
(7 products instead of 8): the weight-side block combos are precomputed
on host; the x/outT-side combos and the 7 post-adds run on DVE, with
product chains interleaved so only ~2 PSUM banks stay live per group.

  P1=A11*B11  P2=A12*B21  P3=S4*B22  P4=A22*T4  P5=S1*T1  P6=S2*T2  P7=S3*T3
  S1=A21+A22  S2=S1-A11  S3=A11-A21  S4=A12-S2      (A side, stationary)
  T1=B12-B11  T2=B22-T1  T3=B22-B12  T4=T2-B21      (B side, moving)
  U1=P1+P6  U2=U1+P7  U3=U1+P5
  C11=P1+P2  C12=U3+P3  C21=U2-P4  C22=U2+P5

Layouts: q/k in [feature, token] (A-side = W); v/y in [token, feature]
(A-side = x/outT on-chip). RoPE via partition-shift DMA + fp16 DVE madd;
attention per (batch, head) with exp bias -1.5 on ACT; softmax sums via
ones-matmul; heads j/j+8 emerge per Strassen group j, pipelined with
the next group's chains.
"""

import numpy as np

f16 = np.float16

# Problem shape (hardcoded per contract)
B, S, H = 32, 256, 2048
NH, HD = 16, 128
N_CORES = 8
B_LOC = B // N_CORES          # 4 batches per core
T = B_LOC * S                 # 1024 tokens per core
P = 128
KH = 8                        # k-tiles per k-half
TH = 512                      # token half
NHF = 1024                    # feature half

_CACHE = {}


def _rope_tables_np(seq_len, head_dim):
    inv_freq = 1.0 / (10000.0 ** (np.arange(0, head_dim, 2, dtype=np.float32) / head_dim))
    t = np.arange(seq_len, dtype=np.float32)
    freqs = np.einsum("i,j->ij", t, inv_freq).astype(np.float32)   # [s, d/2]
    emb = np.concatenate([freqs, freqs], axis=-1)                   # [s, d]
    return np.cos(emb).astype(np.float32), np.sin(emb).astype(np.float32)


def _stat_ops_qk(W):
    """W [2048 n, 2048 k] (A side). -> [128, 8 j, 7*8*128] fp16 lhsT ops."""
    A11, A12 = W[:NHF, :NHF], W[:NHF, NHF:]
    A21, A22 = W[NHF:, :NHF], W[NHF:, NHF:]
    S1 = A21 + A22
    S2 = S1 - A11
    S3 = A11 - A21
    S4 = A12 - S2
    ops = [A11, A12, S4, A22, S1, S2, S3]
    ts = [np.ascontiguousarray(o.T).reshape(KH, P, NHF).transpose(1, 0, 2) for o in ops]
    arr = np.stack(ts, axis=1)                                  # [128, 7, 8kt, 1024n]
    arr = arr.reshape(P, 7, KH, 8, P).transpose(0, 3, 1, 2, 4)  # [128, j, op, kt, 128]
    return np.ascontiguousarray(arr).reshape(P, 8, 7 * KH * P).astype(f16)


def _mov_ops(WT):
    """WT [2048 k, 2048 n] (B side). -> [128, 2 c, 7*8*512] fp16 moving ops."""
    B11, B12 = WT[:NHF, :NHF], WT[:NHF, NHF:]
    B21, B22 = WT[NHF:, :NHF], WT[NHF:, NHF:]
    T1 = B12 - B11
    T2 = B22 - T1
    T3 = B22 - B12
    T4 = T2 - B21
    ops = [B11, B21, B22, T4, T1, T2, T3]
    ts = [o.reshape(KH, P, NHF).transpose(1, 0, 2) for o in ops]    # [128, kt, n]
    arr = np.stack(ts, axis=1)                                  # [128, 7, 8, 1024]
    arr = arr.reshape(P, 7, KH, 2, TH).transpose(0, 3, 1, 2, 4)  # [128, c, op, kt, 512]
    return np.ascontiguousarray(arr).reshape(P, 2, 7 * KH * TH).astype(f16)


def build_nc():
    import concourse.tile as tile
    from concourse import bacc, mybir
    import bass_rust

    AF = bass_rust.ActivationFunctionType
    from concourse.alu_op_type import AluOpType

    f16t = mybir.dt.float16
    f32 = mybir.dt.float32
    ADD, SUB, MUL = AluOpType.add, AluOpType.subtract, AluOpType.mult

    nc = bacc.Bacc("TRN2", target_bir_lowering=False, debug=False, num_devices=N_CORES)

    # x^T quarters: a=(k-lo,t-lo) b=(k-hi,t-lo) c=(k-lo,t-hi) d=(k-hi,t-hi)
    xq_d = {q: nc.dram_tensor(f"x{q}", [P, KH * TH], f16t, kind="ExternalInput").ap()
            for q in "abcd"}
    wq_d = nc.dram_tensor("wq", [P, 8, 7 * KH * P], f16t, kind="ExternalInput").ap()
    wk_d = nc.dram_tensor("wk", [P, 8, 7 * KH * P], f16t, kind="ExternalInput").ap()
    wv_d = nc.dram_tensor("wv", [P, 2, 7 * KH * TH], f16t, kind="ExternalInput").ap()
    wo_d = nc.dram_tensor("wo", [P, 2, 7 * KH * TH], f16t, kind="ExternalInput").ap()
    cosq_d = nc.dram_tensor("cosq", [P, TH], f16t, kind="ExternalInput").ap()
    sinq_d = nc.dram_tensor("sinq", [P, TH], f16t, kind="ExternalInput").ap()
    cosk_d = nc.dram_tensor("cosk", [P, TH], f16t, kind="ExternalInput").ap()
    sink_d = nc.dram_tensor("sink", [P, TH], f16t, kind="ExternalInput").ap()
    ones_d = nc.dram_tensor("ones", [P, P], f16t, kind="ExternalInput").ap()
    y_d = nc.dram_tensor("y", [T, H], f32, kind="ExternalOutput").ap()

    with tile.TileContext(nc) as tc:
        with (
            tc.tile_pool(name="consts", bufs=1) as consts,
            tc.tile_pool(name="otp", bufs=1) as otp,
        ):
            ones_sb = consts.tile([P, P], f16t, name="ones")
            nc.gpsimd.dma_start(ones_sb[:], ones_d)
            ebias = consts.tile([P, 1], f32, name="ebias")
            nc.gpsimd.memset(ebias[:], -1.5)
            tabs = {}
            for nm, d in (("cosq", cosq_d), ("sinq", sinq_d),
                          ("cosk", cosk_d), ("sink", sink_d)):
                tabs[nm] = consts.tile([P, TH], f16t, name=nm)
                nc.gpsimd.dma_start(tabs[nm][:], d)

            mid_cm = tc.tile_pool(name="mid", bufs=1)
            mid = mid_cm.__enter__()
            xabd = tcp = vpool = mid
            xa = xabd.tile([P, KH * TH], f16t, name="xa")
            xb = xabd.tile([P, KH * TH], f16t, name="xb")
            xd = xabd.tile([P, KH * TH], f16t, name="xd")
            nc.sync.dma_start(xa[:], xq_d["a"])
            nc.sync.dma_start(xd[:], xq_d["d"])
            nc.sync.dma_start(xb[:], xq_d["b"])

            tcb = [tcp.tile([P, KH * TH], f16t, name=f"T{i}") for i in range(4)]
            v_sb = vpool.tile([P, 8 * H], f16t, name="v")    # v [t-tile, feat]

            def kslc(tile_, kt, lo, ln):
                return tile_[:, kt * TH + lo: kt * TH + lo + ln]

            def chain_major_phase(nc_pools, stat_list, w_dram, out_cb):
                """One Winograd phase in [token, feature]-output orientation
                (V or O projection): 2 c-chunks x 7 chain-major products x
                4 jj token-slices. out_cb(quad, jj, c, in0, in1, op) emits
                the final post-add (DVE) for that C-quadrant slice."""
                wpool, upool, pspool = nc_pools
                for c in range(2):
                    pv, u1, u2, u3 = {}, {}, {}, {}

                    def chains(i, jj_cb):
                        wt = wpool.tile([P, KH * TH], f16t, name="wt")
                        nc.sync.dma_start(
                            wt[:], w_dram[:, c, (i - 1) * KH * TH:i * KH * TH])
                        for jj in range(4):
                            pt = pspool.tile([P, TH], f32, name="pv")
                            for kt in range(KH):
                                nc.tensor.matmul(
                                    pt[:],
                                    stat_list[i - 1](kt, jj * P, P),
                                    kslc(wt, kt, 0, TH),
                                    start=(kt == 0), stop=(kt == KH - 1),
                                )
                            jj_cb(jj, pt)

                    def p1cb(jj, pt):
                        p1s = upool.tile([P, TH], f32, name="p1s")
                        nc.scalar.activation(p1s[:], pt[:], AF.Copy)
                        pv[(1, jj)] = p1s
                    chains(1, p1cb)

                    def p2cb(jj, pt):
                        out_cb(0, jj, c, pv[(1, jj)][:], pt[:], ADD)
                    chains(2, p2cb)

                    def p6cb(jj, pt):
                        u1[jj] = upool.tile([P, TH], f32, name="u1")
                        nc.vector.tensor_tensor(u1[jj][:], pv[(1, jj)][:], pt[:], ADD)
                    chains(6, p6cb)

                    def p7cb(jj, pt):
                        u2[jj] = upool.tile([P, TH], f32, name="u2")
                        nc.vector.tensor_tensor(u2[jj][:], u1[jj][:], pt[:], ADD)
                    chains(7, p7cb)

                    def p4cb(jj, pt):
                        out_cb(2, jj, c, u2[jj][:], pt[:], SUB)
                    chains(4, p4cb)

                    def p5cb(jj, pt):
                        u3[jj] = upool.tile([P, TH], f32, name="u3")
                        nc.vector.tensor_tensor(u3[jj][:], u1[jj][:], pt[:], ADD)
                        out_cb(3, jj, c, u2[jj][:], pt[:], ADD)
                    chains(5, p5cb)

                    def p3cb(jj, pt):
                        out_cb(1, jj, c, u3[jj][:], pt[:], ADD)
                    chains(3, p3cb)

            # ---------------- V projection (Winograd, chain-major) ----------
            with (
                tc.tile_pool(name="scp", bufs=1) as scp,
                tc.tile_pool(name="wvs", bufs=2) as wvs,
                tc.tile_pool(name="vps", bufs=6, space="PSUM") as vps,
                tc.tile_pool(name="warm", bufs=1, space="PSUM") as warm,
            ):
                # PE warmup (p-state ramp) while input DMAs land
                wps = warm.tile([P, P], f32, name="warm")
                for _ in range(12):
                    nc.tensor.matmul(wps[:], ones_sb[:], ones_sb[:],
                                     start=True, stop=True)

                scb = [scp.tile([P, KH * TH], f16t, name=f"S{i}") for i in range(4)]
                with tc.tile_pool(name="xcq", bufs=1) as xcp:
                    xc = xcp.tile([P, KH * TH], f16t, name="xc")
                    nc.sync.dma_start(xc[:], xq_d["c"])

                    # S-combos (V stationary) + T-combos (Q/K moving)
                    for kt in range(KH):
                        sl = slice(kt * TH, (kt + 1) * TH)
                        nc.vector.tensor_tensor(scb[0][:, sl], xc[:, sl], xd[:, sl], ADD)
                        nc.vector.tensor_tensor(scb[1][:, sl], scb[0][:, sl], xa[:, sl], SUB)
                        nc.vector.tensor_tensor(scb[2][:, sl], xa[:, sl], xc[:, sl], SUB)
                        nc.vector.tensor_tensor(scb[3][:, sl], xb[:, sl], scb[1][:, sl], SUB)
                    for kt in range(KH):
                        sl = slice(kt * TH, (kt + 1) * TH)
                        nc.vector.tensor_tensor(tcb[0][:, sl], xc[:, sl], xa[:, sl], SUB)
                        nc.vector.tensor_tensor(tcb[1][:, sl], xd[:, sl], tcb[0][:, sl], SUB)
                        nc.vector.tensor_tensor(tcb[2][:, sl], xd[:, sl], xc[:, sl], SUB)
                        nc.vector.tensor_tensor(tcb[3][:, sl], tcb[1][:, sl], xb[:, sl], SUB)

                utv_cm = tc.tile_pool(name="utv", bufs=4)
                utv = utv_cm.__enter__()
                vstat = [xa, xb, scb[3], xd, scb[0], scb[1], scb[2]]
                vstat_fn = [lambda kt, lo, ln, t=t_: kslc(t, kt, lo, ln) for t_ in vstat]

                def v_out(quad, jj, c, in0, in1, op):
                    tt = jj + (4 if quad in (2, 3) else 0)
                    col = (c * TH) + (NHF if quad in (1, 3) else 0)
                    nc.vector.tensor_tensor(
                        v_sb[:, tt * H + col: tt * H + col + TH], in0, in1, op)

                chain_major_phase((wvs, utv, vps), vstat_fn, wv_d, v_out)
                utv_cm.__exit__(None, None, None)

            outT = otp.tile([P, NH * T], f16t, name="outT")  # attn out [d, (h, t)]

            # ---------------- Q/K projections + RoPE + attention ------------
            with (
                tc.tile_pool(name="wqs", bufs=6) as wqs,
                tc.tile_pool(name="wks", bufs=6) as wks,
                tc.tile_pool(name="qgp", bufs=2) as qgp,
                tc.tile_pool(name="kgp", bufs=2) as kgp,
                tc.tile_pool(name="qrp", bufs=2) as qrp,
                tc.tile_pool(name="krp", bufs=2) as krp,
                tc.tile_pool(name="rqp", bufs=2) as rqp,
                tc.tile_pool(name="m12", bufs=2) as m12,
                tc.tile_pool(name="utq", bufs=2) as utq,
                tc.tile_pool(name="ebp", bufs=3) as ebp,
                tc.tile_pool(name="rsp", bufs=2) as rsp,
                tc.tile_pool(name="qps", bufs=3, space="PSUM") as qps,
                tc.tile_pool(name="sps", bufs=2, space="PSUM") as sps,
                tc.tile_pool(name="dps", bufs=2, space="PSUM") as dps,
            ):
                qkmov = [xa, xb, xd, tcb[3], tcb[0], tcb[1], tcb[2]]
                pending = []

                def flush_one():
                    if pending:
                        pending.pop(0)()

                def qk_group(w_dram, wpool, j, gtile):
                    """Group-major Winograd group j -> heads j, j+8 into gtile
                    (flat [128, 2*1024])."""
                    pv, ut = {}, {}

                    def chain(i):
                        wt = wpool.tile([P, KH * P], f16t, name="wt")
                        nc.sync.dma_start(
                            wt[:], w_dram[:, j, (i - 1) * KH * P:i * KH * P])
                        pt = qps.tile([P, TH], f32, name="pq")
                        for kt in range(KH):
                            nc.tensor.matmul(
                                pt[:],
                                wt[:, kt * P:(kt + 1) * P],
                                kslc(qkmov[i - 1], kt, 0, TH),
                                start=(kt == 0), stop=(kt == KH - 1),
                            )
                        pv[i] = pt
                        flush_one()

                    def out(quad):
                        hh = 1 if quad in (2, 3) else 0
                        lo = TH if quad in (1, 3) else 0
                        return gtile[:, hh * T + lo: hh * T + lo + TH]

                    chain(1)
                    p1s = utq.tile([P, TH], f32, name="p1s")
                    nc.scalar.activation(p1s[:], pv[1][:], AF.Copy)
                    chain(2)
                    nc.vector.tensor_tensor(out(0), p1s[:], pv[2][:], ADD)
                    chain(6)
                    ut[1] = utq.tile([P, TH], f32, name="u1")
                    nc.vector.tensor_tensor(ut[1][:], p1s[:], pv[6][:], ADD)
                    chain(7)
                    ut[2] = utq.tile([P, TH], f32, name="u2")
                    nc.vector.tensor_tensor(ut[2][:], ut[1][:], pv[7][:], ADD)
                    chain(4)
                    nc.vector.tensor_tensor(out(2), ut[2][:], pv[4][:], SUB)
                    chain(5)
                    ut[3] = utq.tile([P, TH], f32, name="u3")
                    nc.vector.tensor_tensor(ut[3][:], ut[1][:], pv[5][:], ADD)
                    nc.vector.tensor_tensor(out(3), ut[2][:], pv[5][:], ADD)
                    chain(3)
                    nc.vector.tensor_tensor(out(1), ut[3][:], pv[3][:], ADD)

                def rope(gtile, rtile, cos_sb, sin_sb):
                    HH = P // 2
                    for hh in range(2):
                        for ts in range(2):
                            sl = slice(hh * T + ts * TH, hh * T + (ts + 1) * TH)
                            src = gtile[:, sl]
                            rq = rqp.tile([P, TH], f16t, name="rq")
                            nc.sync.dma_start(rq[0:HH, :], src[HH:P, :])
                            nc.sync.dma_start(rq[HH:P, :], src[0:HH, :])
                            m1 = m12.tile([P, TH], f16t, name="m1")
                            nc.vector.tensor_tensor(m1[:], src, cos_sb[:], MUL)
                            m2 = m12.tile([P, TH], f16t, name="m2")
                            nc.vector.tensor_tensor(m2[:], rq[:], sin_sb[:], MUL)
                            nc.vector.tensor_tensor(rtile[:, sl], m1[:], m2[:], ADD)

                def attn_unit(b, hh, h, qr, kr):
                    def emit():
                        bq = slice(hh * T + b * S, hh * T + (b + 1) * S)
                        pS = sps.tile([P, 2, S], f32, name="pS")
                        for sk in range(2):
                            nc.tensor.matmul(
                                pS[:, sk],
                                kr[:, hh * T + b * S + sk * P:
                                   hh * T + b * S + (sk + 1) * P],
                                qr[:, bq],
                                start=True, stop=True,
                            )
                        ebf = ebp.tile([P, 2, S], f16t, name="ebf")
                        nc.scalar.activation(ebf[:], pS[:], AF.Exp, bias=ebias[:])
                        dp = dps.tile([P, 2, S], f32, name="dp")
                        for sk in range(2):
                            nc.tensor.matmul(dp[:, 0], ones_sb[:], ebf[:, sk],
                                             start=(sk == 0), stop=(sk == 1))
                        rsb = rsp.tile([P, S], f32, name="rsb")
                        nc.vector.reciprocal_approx_fast(rsb[:], dp[:, 0])
                        for sk in range(2):
                            nc.tensor.matmul(
                                dp[:, 1],
                                v_sb[:, (2 * b + sk) * H + h * P:
                                     (2 * b + sk) * H + (h + 1) * P],
                                ebf[:, sk],
                                start=(sk == 0), stop=(sk == 1),
                            )
                        nc.vector.tensor_tensor(
                            outT[:, h * T + b * S: h * T + (b + 1) * S],
                            dp[:, 1], rsb[:], MUL)
                    return emit

                def qk_phase():
                    for j in range(8):
                        qg = qgp.tile([P, 2 * T], f16t, name="qg")
                        kg = kgp.tile([P, 2 * T], f16t, name="kg")
                        qk_group(wq_d, wqs, j, qg)
                        qk_group(wk_d, wks, j, kg)
                        qr = qrp.tile([P, 2 * T], f16t, name="qr")
                        kr = krp.tile([P, 2 * T], f16t, name="kr")
                        rope(qg, qr, tabs["cosq"], tabs["sinq"])
                        rope(kg, kr, tabs["cosk"], tabs["sink"])
                        for b in range(B_LOC):
                            for hh in range(2):
                                pending.append(attn_unit(b, hh, j + 8 * hh, qr, kr))
                    for e in pending:
                        e()

                qk_phase()

            mid_cm.__exit__(None, None, None)

            # ---------------- output projection (Winograd, chain-major) -----
            with (
                tc.tile_pool(name="socp", bufs=1) as socp,
                tc.tile_pool(name="wos", bufs=3) as wos,
                tc.tile_pool(name="uto", bufs=5) as uto,
                tc.tile_pool(name="ysb", bufs=6) as ysb,
                tc.tile_pool(name="ops", bufs=6, space="PSUM") as opsum,
            ):
                def ob(kt, th):
                    return outT[:, kt * T + th * TH: kt * T + th * TH + TH]

                socb = [socp.tile([P, KH * TH], f16t, name=f"So{i}") for i in range(4)]
                for kt in range(KH):
                    sl = slice(kt * TH, (kt + 1) * TH)
                    nc.vector.tensor_tensor(socb[0][:, sl], ob(kt, 1), ob(kt + 8, 1), ADD)
                    nc.vector.tensor_tensor(socb[1][:, sl], socb[0][:, sl], ob(kt, 0), SUB)
                    nc.vector.tensor_tensor(socb[2][:, sl], ob(kt, 0), ob(kt, 1), SUB)
                    nc.vector.tensor_tensor(socb[3][:, sl], ob(kt + 8, 0), socb[1][:, sl], SUB)

                def oslc(th, k_hi):
                    def fn(kt, lo, ln):
                        base = (kt + (8 if k_hi else 0)) * T + th * TH
                        return outT[:, base + lo: base + lo + ln]
                    return fn

                ostat = [
                    oslc(0, False), oslc(0, True),
                    lambda kt, lo, ln: kslc(socb[3], kt, lo, ln),
                    oslc(1, True),
                    lambda kt, lo, ln: kslc(socb[0], kt, lo, ln),
                    lambda kt, lo, ln: kslc(socb[1], kt, lo, ln),
                    lambda kt, lo, ln: kslc(socb[2], kt, lo, ln),
                ]

                def y_out(quad, jj, c, in0, in1, op):
                    yt = ysb.tile([P, TH], f32, name="yt")
                    nc.vector.tensor_tensor(yt[:], in0, in1, op)
                    r0 = (jj + (4 if quad in (2, 3) else 0)) * P
                    col = (c * TH) + (NHF if quad in (1, 3) else 0)
                    nc.sync.dma_start(y_d[r0:r0 + P, col:col + TH], yt[:])

                chain_major_phase((wos, uto, opsum), ostat, wo_d, y_out)

    nc.compile()
    return nc


def _host_prep(hidden_states, Wq, Wk, Wv, Wo):
    x = np.asarray(hidden_states, dtype=np.float32).reshape(B * S, H)
    Wq = np.asarray(Wq, dtype=np.float32)
    Wk = np.asarray(Wk, dtype=np.float32)
    WvT = np.ascontiguousarray(np.asarray(Wv, dtype=np.float32).T)
    WoT = np.ascontiguousarray(np.asarray(Wo, dtype=np.float32).T)

    wq_h = _stat_ops_qk(Wq)
    wk_h = _stat_ops_qk(Wk)
    wv_h = _mov_ops(WvT)
    wo_h = _mov_ops(WoT)

    cos, sin = _rope_tables_np(S, HD)              # [s, d]
    cosT = np.ascontiguousarray(cos.T)             # [d, s]
    sinT = np.ascontiguousarray(sin.T)
    sgn = np.where(np.arange(HD) < HD // 2, -1.0, 1.0).astype(np.float32)[:, None]
    sinT = sinT * sgn
    scale = np.float32(HD ** -0.5)
    cosq = np.tile(cosT * scale, (1, 2)).astype(f16)   # [128, 512]
    sinq = np.tile(sinT * scale, (1, 2)).astype(f16)
    cosk = np.tile(cosT, (1, 2)).astype(f16)
    sink = np.tile(sinT, (1, 2)).astype(f16)
    ones = np.ones((P, P), f16)

    shared = {
        "wq": wq_h, "wk": wk_h, "wv": wv_h, "wo": wo_h,
        "cosq": cosq, "sinq": sinq, "cosk": cosk, "sink": sink,
        "ones": ones,
    }
    in_maps = []
    for c in range(N_CORES):
        xc = x[c * T:(c + 1) * T]                       # [T, H]
        xT = np.ascontiguousarray(xc.T)                 # [2048, 1024]
        xt = xT.reshape(16, P, T).transpose(1, 0, 2)    # [128, 16 kt, 1024]
        quarters = {
            "xa": xt[:, 0:KH, 0:TH], "xb": xt[:, KH:16, 0:TH],
            "xc": xt[:, 0:KH, TH:T], "xd": xt[:, KH:16, TH:T],
        }
        qmaps = {k: np.ascontiguousarray(v).reshape(P, KH * TH).astype(f16)
                 for k, v in quarters.items()}
        in_maps.append({**qmaps, **shared})
    return in_maps


def _run(hidden_states, Wq, Wk, Wv, Wo, **spmd_kwargs):
    from concourse import bass_utils

    if "nc" not in _CACHE:
        _CACHE["nc"] = build_nc()
    nc = _CACHE["nc"]

    in_maps = _host_prep(hidden_states, Wq, Wk, Wv, Wo)
    res = bass_utils.run_bass_kernel_spmd(
        nc, in_maps, core_ids=list(range(N_CORES)), **spmd_kwargs
    )
    y = np.concatenate([r["y"] for r in res.results], axis=0)  # [B*S, H]
    return y.reshape(B, S, H).astype(np.float32), res


def kernel(hidden_states, Wq, Wk, Wv, Wo):
    y, _ = _run(hidden_states, Wq, Wk, Wv, Wo)
    return y


def run_traced(hidden_states, Wq, Wk, Wv, Wo):
    """Like kernel(), but captures an NTFF profile; returns (y, results)."""
    return _run(hidden_states, Wq, Wk, Wv, Wo, trace=True)
